# revision 8
# baseline (speedup 1.0000x reference)
"""Trainium2 Bass kernel for nn_ConvDecoder (RBF set-conv decoder).

Reference computation:
    rbf[b,t,g] = exp(-0.5*((x_grid[g]-x_target[b,t])/exp(sigma))^2)
    z[b,t,c]   = sum_g rbf[b,t,g] * r[b,c,g]
    out        = z @ W + b_lin                       # (4, 4096, 2)

The Gaussian kernel matrix K_tg is numerically low rank; a Nystrom
factorization through m=32 uniform anchors u (host-folded pinv(Kuu)
into bounded cardinal functions EguM = K_gu @ pinv(Kuu)) gives

    K_tg ~= E_tu @ EguM^T        (error ~5e-4 at fp16 storage)

Sharding: core k = (batch b = k//2, grid half gh = k%2). Each core
contracts its half of the grid and produces a PARTIAL output for all
4096 targets of its batch; the host sums the two halves and adds b_lin.
This halves per-core HBM traffic vs a target-split (only 0.75 MB/core).

Per-core device pipeline (T=4096 targets, 4096 grid rows):
  args = lhsT.T @ rhs   K=28 fp16 matmul -> (128, 1024) PSUM fp32
         (4 target-quarters packed on partitions: row 32*jq+u covers
          anchor u / target quarter jq; fp32 accuracy recovered via
          hi/lo-split fp16 rows, since fp16 products accumulate
          exactly in fp32 PSUM)
  eut  = exp(args)      one ACT call -> (128, 1024) f16
  S^T  = sum_j rt_j^T @ egu_j   32 accumulating K=128 matmuls -> (64, 32)
  P    = S @ W          4 matmuls into block-diagonal (128, 8) layout
  out  = eut-chunk^T @ P_blk    8 K=128 matmuls -> (128, 64) -> DMA

All big operands ride ONE merged DRAM tensor (128, 32, 96) f16
(egu | rt interleaved per 128-row grid chunk) in 2 DMA slices; all
small operands ride ONE (64, 1162) f16 const tensor.
"""

import sys

if "/opt/trn_rl_repo" not in sys.path:
    sys.path.insert(0, "/opt/trn_rl_repo")

import numpy as np

# Problem shapes (hardcoded per spec)
B = 4          # batch
C = 64         # conv channels
G = 8192       # grid points
TFULL = 4096   # targets per batch (all handled by each core)
NCORES = 8
GH = G // 2            # grid rows per core
JC = GH // 128         # 32 grid chunks of 128
M = 32                 # Nystrom anchors
NQ = 4                 # target quarters packed on partitions (4*32=128)
TQ = TFULL // NQ       # 1024 targets per quarter = eut cols
KROWS = 7 * NQ         # 28 fp16 arg rows (hi/lo split)
OUT_CH = 2
CCHUNK = TQ // 128     # 8 final-contraction chunks
MARGIN = 2.0           # anchor span margin in units of s

# big-tensor DMA slice boundaries (chunk units); last slice carries the
# extra W chunk so the post-DMA S^T tail is short
BIG_SLICES = (0, 16, 24, 28, JC + 1)
CST_COLS = TQ + 128    # rhs | lhsT

_PROGRAM = None


def _build_program():
    import concourse.bass as bass
    import concourse.tile as tile
    from concourse import bacc, mybir

    f32 = mybir.dt.float32
    f16 = mybir.dt.float16
    Exp = mybir.ActivationFunctionType.Exp

    nc = bacc.Bacc(None, target_bir_lowering=False)
    dr_big = nc.dram_tensor("big", [128, JC + 1, M + C], f16, kind="ExternalInput")
    dr_cst = nc.dram_tensor("cst", [KROWS, CST_COLS], f16, kind="ExternalInput")
    dr_out = nc.dram_tensor("out", [128, CCHUNK * NQ * OUT_CH], f32,
                            kind="ExternalOutput")

    with tile.TileContext(nc) as tc:
        with (
            tc.tile_pool(name="sb", bufs=1) as sbp,
            tc.tile_pool(name="ps", bufs=1, space=bass.MemorySpace.PSUM) as psp,
        ):
            # ---- DMAs in (all on the sync ring; cst first, it gates exp) ----
            cst = sbp.tile([KROWS, CST_COLS], f16, tag="cst")
            nc.sync.dma_start(cst[:], dr_cst[:])

            big = sbp.tile([128, JC + 1, M + C], f16, tag="big")
            for q in range(len(BIG_SLICES) - 1):
                j0, j1 = BIG_SLICES[q], BIG_SLICES[q + 1]
                nc.sync.dma_start(big[:, j0:j1, :], dr_big[:, j0:j1, :])

            a_rhs = cst[0:KROWS, 0:TQ]
            a_lhsT = cst[0:KROWS, TQ : TQ + 128]
            # W blocks live in big's extra chunk: wa8_jq = cols jq*8..jq*8+8
            # of chunk JC (rows 0:64), W pre-shifted to cols 2jq of its block

            # ---- eut = exp(args), 4 quarters packed on partitions ----
            args_ps = psp.tile([128, TQ], f32, tag="args")
            for n in range(TQ // 512):
                nc.tensor.matmul(
                    args_ps[:, n * 512 : (n + 1) * 512],
                    a_lhsT,
                    a_rhs[:, n * 512 : (n + 1) * 512],
                    start=True,
                    stop=True,
                )
            eut = sbp.tile([128, TQ], f16, tag="eut")
            nc.scalar.activation(eut[:], args_ps[:], Exp)

            # ---- S^T[c,u] accumulated over 32 grid chunks ----
            st_ps = psp.tile([C, M], f32, tag="st")
            for j in range(JC):
                nc.tensor.matmul(
                    st_ps[:],
                    big[:, j, M : M + C],
                    big[:, j, 0:M],
                    start=(j == 0),
                    stop=(j == JC - 1),
                )
            st_sb = sbp.tile([C, M], f16, tag="st")
            nc.vector.tensor_copy(st_sb[:], st_ps[:])

            # ---- P = S @ W into block-diagonal (128, 8) f16 ----
            # each matmul writes a full (32, 8) partition stripe (its W block
            # is pre-shifted to cols 2jq, zeros elsewhere) -> no memset and a
            # single PSUM->SBUF copy
            p_blk = sbp.tile([128, NQ * OUT_CH], f16, tag="pblk")
            pb_ps = psp.tile([128, NQ * OUT_CH], f32, tag="pb")
            NW = NQ * OUT_CH
            for jq in range(NQ):
                nc.tensor.matmul(
                    pb_ps[32 * jq : 32 * (jq + 1), :],
                    st_sb[:],
                    big[0:C, JC, jq * NW : (jq + 1) * NW],
                    start=True,
                    stop=True,
                    tile_position=(0, 32 * jq),
                )
            nc.vector.tensor_copy(p_blk[:], pb_ps[:])

            # ---- partial out: (128, 8) per 128-col eut chunk ----
            v_ps = psp.tile([128, CCHUNK * NQ * OUT_CH], f32, tag="v")
            for cch in range(CCHUNK):
                nc.tensor.matmul(
                    v_ps[:, cch * NW : (cch + 1) * NW],
                    eut[:, cch * 128 : (cch + 1) * 128],
                    p_blk[:],
                    start=True,
                    stop=True,
                )
            out_sb = sbp.tile([128, CCHUNK * NW], f32, tag="o")
            nc.vector.tensor_copy(out_sb[:], v_ps[:])
            nc.scalar.dma_start(dr_out[:], out_sb[:])

    nc.compile()
    return nc


def _get_program():
    global _PROGRAM
    if _PROGRAM is None:
        _PROGRAM = _build_program()
    return _PROGRAM


def _f16(a):
    return a.astype(np.float16)


def kernel(r, x_context, y_context, x_target, x_grid, sigma, W, b_lin):
    from concourse.bass_utils import run_bass_kernel_spmd

    r = np.asarray(r, dtype=np.float32)
    xt_all = np.asarray(x_target, dtype=np.float64)[..., 0]       # (B, TFULL)
    xg = np.asarray(x_grid, dtype=np.float64)[:, 0]               # (G,)
    s = float(np.exp(np.float64(np.asarray(sigma).reshape(-1)[0])))
    W64 = np.asarray(W, dtype=np.float64)
    b_lin = np.asarray(b_lin, dtype=np.float64)

    # ---- host-side Nystrom factor prep (O(G*M), fp64) ----
    lo = min(xg.min(), xt_all.min()) - MARGIN * s
    hi = max(xg.max(), xt_all.max()) + MARGIN * s
    u = np.linspace(lo, hi, M)
    inv_s2 = 1.0 / (s * s)
    Kuu = np.exp(-0.5 * ((u[:, None] - u[None, :]) / s) ** 2)
    Minv = np.linalg.pinv(Kuu, rcond=1e-10)
    EguM = np.exp(-0.5 * ((xg[:, None] - u[None, :]) / s) ** 2) @ Minv  # (G, M)
    egu16 = _f16(EguM)

    # anchor-side hi/lo rows (shared across batches)
    uc = u * inv_s2
    uch = _f16(uc)
    ucl = _f16(uc - uch.astype(np.float64))
    a_u = -0.5 * u * u * inv_s2
    ah = _f16(a_u)
    al = _f16(a_u - ah.astype(np.float64))

    cst_by_batch = []
    for b in range(B):
        x = xt_all[b]
        bt = -0.5 * x * x * inv_s2
        xh = _f16(x)
        xl = _f16(x - xh.astype(np.float64))
        bh = _f16(bt)
        bl = _f16(bt - bh.astype(np.float64))
        cst = np.zeros((KROWS, CST_COLS), dtype=np.float16)
        for jq in range(NQ):
            base = 7 * jq
            sl = slice(jq * TQ, (jq + 1) * TQ)
            # rhs region: cols 0:TQ
            cst[base + 0, 0:TQ] = xh[sl]
            cst[base + 1, 0:TQ] = xl[sl]
            cst[base + 2, 0:TQ] = xh[sl]
            cst[base + 3, 0:TQ] = bh[sl]
            cst[base + 4, 0:TQ] = bl[sl]
            cst[base + 5, 0:TQ] = 1.0
            cst[base + 6, 0:TQ] = 1.0
            # lhsT region: cols TQ:TQ+128, partition block jq
            pcols = slice(TQ + 32 * jq, TQ + 32 * (jq + 1))
            cst[base + 0, pcols] = uch
            cst[base + 1, pcols] = uch
            cst[base + 2, pcols] = ucl
            cst[base + 3, pcols] = 1.0
            cst[base + 4, pcols] = 1.0
            cst[base + 5, pcols] = ah
            cst[base + 6, pcols] = al
        cst_by_batch.append(np.ascontiguousarray(cst))

    # W chunk: block jq occupies cols jq*8..(jq+1)*8, W at local cols 2jq
    wchunk = np.zeros((128, M + C), dtype=np.float16)
    W16 = _f16(W64)
    for jq in range(NQ):
        wchunk[0:C, jq * NQ * OUT_CH + 2 * jq : jq * NQ * OUT_CH + 2 * jq + 2] = W16

    in_maps = []
    for k in range(NCORES):
        b, gh = divmod(k, 2)
        gsl = slice(gh * GH, (gh + 1) * GH)
        big = np.empty((128, JC + 1, M + C), dtype=np.float16)
        big[:, 0:JC, 0:M] = egu16[gsl].reshape(JC, 128, M).transpose(1, 0, 2)
        big[:, 0:JC, M:] = _f16(r[b].T[gsl]).reshape(JC, 128, C).transpose(1, 0, 2)
        big[:, JC, :] = wchunk
        in_maps.append({"big": np.ascontiguousarray(big), "cst": cst_by_batch[b]})

    nc = _get_program()
    res = run_bass_kernel_spmd(nc, in_maps, core_ids=list(range(NCORES)))

    out = np.empty((B, TFULL, OUT_CH), dtype=np.float32)
    for b in range(B):
        acc = None
        for gh in range(2):
            v = res.results[2 * b + gh]["out"].reshape(128, CCHUNK, NQ, OUT_CH)
            part = v.transpose(2, 1, 0, 3).reshape(TFULL, OUT_CH)
            acc = part if acc is None else acc + part
        out[b] = acc
    out += b_lin.astype(np.float32)[None, None, :]
    return out
